# revision 12
# baseline (speedup 1.0000x reference)
"""Causal self-attention (RoPE) Trainium2 Bass kernel, SPMD over 8 NeuronCores.

Sharding: data-parallel over batch (B=2) x tensor-parallel over heads
(16 heads -> 4 heads per core).  core c handles batch c//4, heads
4*(c%4) .. 4*(c%4)+3.  Each core computes its heads' attention output and a
partial out@Wo contribution ([S, H] in f16); the host sums the 4 partials
per batch.

Device pipeline per core (transposed-scores formulation, bf16 attention):
  1. QKV projection (fp32r) from hidden^T; RoPE (f32) on q,k; q,k cast to
     bf16 and PE-transposed (4 tiles into one PSUM bank, single bulk copy);
     v stored natural bf16 with 64 appended ones-columns (V'), so the AV
     matmul replicates the softmax denominator on partitions 64..127 free
     of charge (no partition broadcast needed for normalization).
  2. 512-column q-passes: scores^T[k,q] = K Q^T (bf16) -> exp -> causal tri
     mask on diagonal tiles -> O'^T[0:128, q] = V'^T P~^T accumulated in a
     single PSUM bank per pass; normalization = reciprocal of rows 64:128
     times rows 0:64 (two DVE ops, PSUM-direct).
  3. O^T normalized into on_t (bf16) -> partial = O_norm @ Wo (bf16) in
     512-wide chunks -> f16 partial out, DMA'd on the DVE queue.
"""

import sys
import numpy as np

for _p in ("/opt/trn_rl_repo", "/root/.axon_site/_ro/trn_rl_repo"):
    if _p not in sys.path:
        sys.path.append(_p)

import ml_dtypes
import concourse.bacc as bacc
from concourse import mybir
from concourse.tile import TileContext
from concourse.bass_utils import run_bass_kernel_spmd

F32 = mybir.dt.float32
F32R = mybir.dt.float32r
BF16 = mybir.dt.bfloat16
F16 = mybir.dt.float16
EXP = mybir.ActivationFunctionType.Exp

NUM_HEADS = 16
HEAD_DIM = 64
ROPE_BASE = 160000.0
N_CORES = 8


def build_nc(S, H, HL, debug=False):
    """Build the SPMD Bass program.

    S: sequence length; H: hidden size; HL: heads per core (local).
    """
    DL = HL * HEAD_DIM          # local channels (256)
    NI = H // 128               # contraction tiles for projections (8)
    NS = S // 128               # sequence tiles (16)
    CT = max(DL // 128, 1)      # channel tiles (2)
    NQ = S // 512               # 512-wide q passes per head (4)
    scale = HEAD_DIM ** -0.5
    assert DL % 128 == 0 and H % 128 == 0 and S % 1024 == 0

    nc = bacc.Bacc("TRN2", target_bir_lowering=False, debug=False,
                   num_devices=N_CORES)
    hT_d = nc.declare_dram_parameter("hT", [H, S], F32, isOutput=False)
    wq_d = nc.declare_dram_parameter("wq", [H, DL], F32, isOutput=False)
    wk_d = nc.declare_dram_parameter("wk", [H, DL], F32, isOutput=False)
    wv_d = nc.declare_dram_parameter("wv", [H, DL], F32, isOutput=False)
    wo_d = nc.declare_dram_parameter("wo", [DL, H], BF16, isOutput=False)
    cs_d = nc.declare_dram_parameter("cs", [S, 96], F32, isOutput=False)
    tri_d = nc.declare_dram_parameter("tri", [128, 128], BF16, isOutput=False)
    id_d = nc.declare_dram_parameter("ident", [128, 128], BF16, isOutput=False)
    out_d = nc.declare_dram_parameter("part", [S, H], F16, isOutput=True)

    with TileContext(nc) as tc:
        with (
            tc.tile_pool(name="w", bufs=1) as w_pool,
            tc.tile_pool(name="persist", bufs=1) as pers,
            tc.tile_pool(name="hstream", bufs=3) as hs_pool,
            tc.tile_pool(name="xall", bufs=3) as xa_pool,
            tc.tile_pool(name="rope", bufs=3) as rope_pool,
            tc.tile_pool(name="qksb", bufs=3) as qk_pool,
            tc.tile_pool(name="psb", bufs=6) as p_pool,
            tc.tile_pool(name="norm", bufs=3) as n_pool,
            tc.tile_pool(name="osb", bufs=3) as o_pool,
            tc.tile_pool(name="ps_sc", bufs=3, space="PSUM") as ps_sc,
            tc.tile_pool(name="ps_o", bufs=2, space="PSUM") as ps_o,
            tc.tile_pool(name="ps_x", bufs=3, space="PSUM") as ps_x,
        ):
            # --- weights / constants (resident) ---
            wq_t = w_pool.tile([128, NI, DL], F32R)
            wk_t = w_pool.tile([128, NI, DL], F32R)
            wv_t = w_pool.tile([128, NI, DL], F32R)
            wo_t = w_pool.tile([128, CT, H], BF16)
            cs_t = w_pool.tile([128, NS, 96], F32)
            tri_t = w_pool.tile([128, 128], BF16)
            id_t = w_pool.tile([128, 128], BF16)

            NIH = NI // 2
            # weight streams split across both HWDGE queues so the first
            # projection groups aren't gated on one serialized queue:
            # ACT carries wq/wv (q and v groups run first), SP carries wk
            # (behind the first h tile).  tri/wo are deferred below.
            wq_r = wq_d[:].rearrange("(t p) d -> p t d", p=128).bitcast(F32R)
            wv_r = wv_d[:].rearrange("(t p) d -> p t d", p=128).bitcast(F32R)
            wk_r = wk_d[:].rearrange("(t p) d -> p t d", p=128).bitcast(F32R)
            nc.scalar.dma_start(out=wq_t[:, 0:NIH, :], in_=wq_r[:, 0:NIH, :])
            nc.scalar.dma_start(out=wq_t[:, NIH:, :], in_=wq_r[:, NIH:, :])
            nc.scalar.dma_start(out=wv_t[:, 0:NIH, :], in_=wv_r[:, 0:NIH, :])
            nc.scalar.dma_start(out=wv_t[:, NIH:, :], in_=wv_r[:, NIH:, :])
            nc.scalar.dma_start(
                out=cs_t, in_=cs_d[:].rearrange("(t p) d -> p t d", p=128))
            nc.scalar.dma_start(out=id_t, in_=id_d[:])

            # persistent activations
            qkT = pers.tile([128, 2, CT, S], BF16)   # [d, q/k, ct, s]
            vv = pers.tile([128, NS, HL, 128], BF16)  # v cols 0:64, ones 64:128
            on_t = pers.tile([128, CT, S], BF16)
            nc.gpsimd.memset(vv[:, :, :, 64:128], 1.0)

            hT_r = hT_d[:].rearrange("(t p) s -> p t s", p=128).bitcast(F32R)

            # ---------------- deferred-emission queues ----------------
            tq = []   # pending transpose closures (phase 1)
            avq = []  # pending AV closures (phase 2)

            def flush(qu, keep=0):
                while len(qu) > keep:
                    qu.pop(0)()

            # ---------------- phase 2 ----------------
            o_tiles = {}

            def seg_for(h, t, hq, first=False, last=False):
                """scores+exp+mask for k-tile t against q in
                [max(128t, 512*hq), 512*(hq+1)); AV deferred."""
                sg = hq * 512
                k0 = t * 128
                o0 = max(k0 - sg, 0)
                if first:
                    assert o0 == 0, "first seg of a pass must cover the q-range"
                base = (h % 2) * 64
                ct = h // 2
                kT = qkT[base:base + 64, 1, ct, k0:k0 + 128]
                qT = qkT[base:base + 64, 0, ct, sg + o0:sg + 512]
                if (h, hq) not in o_tiles:
                    o_tiles[(h, hq)] = ps_o.tile([128, 512], F32, tag="o",
                                                 name=f"o_{h}_{hq}")
                o_ps = o_tiles[(h, hq)]
                sc = ps_sc.tile([128, 512], F32, tag="sc")
                nc.tensor.matmul(sc[:, o0:512], kT, qT, start=True, stop=True)
                flush(avq, keep=3)
                p = p_pool.tile([128, 512], BF16, tag="p")
                nc.scalar.activation(p[:, o0:512], sc[:, o0:512], EXP,
                                     scale=scale)
                if k0 >= sg:  # diagonal tile: causal mask
                    nc.vector.tensor_mul(p[:, o0:o0 + 128], p[:, o0:o0 + 128],
                                         tri_t)

                def av():
                    nc.tensor.matmul(o_ps[:, o0:512], vv[:, t, h, :],
                                     p[:, o0:512], start=first, stop=last,
                                     skip_group_check=True)
                avq.append(av)

            def norm_for(h, hq):
                flush(avq)
                sg = hq * 512
                base = (h % 2) * 64
                ct = h // 2
                o_ps = o_tiles.pop((h, hq))
                # rows 64:128 of O' hold the softmax denominator replicated
                # (ones-columns of V'); normalize PSUM-direct: two DVE ops
                r_sb = n_pool.tile([64, 512], F32, tag="r")
                nc.vector.reciprocal(r_sb, o_ps[64:128, :])
                nc.vector.tensor_mul(on_t[base:base + 64, ct, sg:sg + 512],
                                     o_ps[0:64, :], r_sb)

            # ---------------- phase 1 ----------------
            def emit_p1(st, split_h=False, after_h=None):
                s0 = st * 128
                h_t = hs_pool.tile([128, NI, 128], F32R, tag="h")
                if split_h:
                    nc.sync.dma_start(out=h_t[:, 0:NIH, :],
                                      in_=hT_r[:, 0:NIH, s0:s0 + 128])
                    nc.sync.dma_start(out=h_t[:, NIH:, :],
                                      in_=hT_r[:, NIH:, s0:s0 + 128])
                else:
                    nc.sync.dma_start(out=h_t, in_=hT_r[:, :, s0:s0 + 128])
                if after_h is not None:
                    after_h()
                qk_ps = ps_x.tile([128, 512], F32, tag="px")
                v_ps = ps_x.tile([128, 512], F32, tag="px")
                for w_t, ps, col in ((wq_t, qk_ps, 0), (wv_t, v_ps, 0),
                                     (wk_t, qk_ps, 256)):
                    for i in range(NI):
                        nc.tensor.matmul(ps[:, col:col + DL], h_t[:, i, :],
                                         w_t[:, i, :],
                                         start=(i == 0), stop=(i == NI - 1))
                flush(tq)

                # split drain: q-half on DVE (shorter chain to q-rope),
                # k-half on ACT; v straight to vv on Pool
                x_all = xa_pool.tile([128, 512], F32, tag="xa")
                nc.vector.tensor_copy(x_all[:, 0:DL], qk_ps[:, 0:DL])
                nc.scalar.copy(x_all[:, DL:], qk_ps[:, DL:])
                nc.gpsimd.tensor_copy(
                    vv[:, st, :, 0:64],
                    v_ps[:, 0:DL].rearrange("p (h d) -> p h d", d=64))

                cosb = cs_t[:, st, 0:32].unsqueeze(1).broadcast_to(
                    [128, 2 * HL, 32])
                sinmb = cs_t[:, st, 32:64].unsqueeze(1).broadcast_to(
                    [128, HL, 32])
                sinpb = cs_t[:, st, 64:96].unsqueeze(1).broadcast_to(
                    [128, HL, 32])
                x_sb = {}
                for qk in (0, 1):
                    xsrc = x_all[:, qk * DL:(qk + 1) * DL]
                    x4 = xsrc.rearrange("p (h two d) -> p h two d", two=2,
                                        d=32)
                    a_t = rope_pool.tile([128, DL], F32, tag="ra")
                    nc.vector.tensor_mul(
                        a_t.rearrange("p (r d) -> p r d", d=32),
                        xsrc.rearrange("p (r d) -> p r d", d=32), cosb)
                    b_t = rope_pool.tile([128, DL], F32, tag="rb")
                    b4 = b_t.rearrange("p (h two d) -> p h two d", two=2,
                                       d=32)
                    nc.vector.tensor_mul(b4[:, :, 0, :], x4[:, :, 1, :],
                                         sinmb)
                    nc.vector.tensor_mul(b4[:, :, 1, :], x4[:, :, 0, :],
                                         sinpb)
                    xs = qk_pool.tile([128, DL], BF16, tag=f"x{qk}")
                    nc.gpsimd.tensor_add(xs, a_t, b_t)
                    x_sb[qk] = xs

                def transposes():
                    t_ps = ps_x.tile([128, 1024], BF16, tag="px")
                    for qk in (0, 1):
                        for ctp in range(CT):
                            idx = qk * CT + ctp
                            nc.tensor.transpose(
                                t_ps[:, idx * 128:(idx + 1) * 128],
                                x_sb[qk][:, ctp * 128:(ctp + 1) * 128],
                                id_t)
                    nc.vector.tensor_copy(
                        qkT[:, :, :, s0:s0 + 128],
                        t_ps[:, 0:512].rearrange("p (a b s) -> p a b s",
                                                 a=2, b=CT))
                tq.append(transposes)

            # ---------------- phase 3 ----------------
            def emit_p3(st):
                s0 = st * 128
                out_sb = o_pool.tile([128, H], F16, tag="out")
                for oc, ceng in ((0, nc.gpsimd), (512, nc.vector)):
                    ps = ps_x.tile([128, 512], F32, tag="px")
                    for ctp in range(CT):
                        nc.tensor.matmul(ps, on_t[:, ctp, s0:s0 + 128],
                                         wo_t[:, ctp, oc:oc + 512],
                                         start=(ctp == 0),
                                         stop=(ctp == CT - 1))
                    ceng.tensor_copy(out_sb[:, oc:oc + 512], ps)
                nc.sync.dma_start(out=out_d[s0:s0 + 128, :], in_=out_sb)

            # ---------------- orchestration ----------------
            def pass_k_order(hq):
                first = 4 * hq
                return ([first] + list(range(first + 1, 4 * (hq + 1))) +
                        list(range(first - 1, -1, -1)))

            def emit_pass(h, hq):
                ts = pass_k_order(hq)
                for j, t in enumerate(ts):
                    seg_for(h, t, hq, first=(j == 0), last=(j == len(ts) - 1))
                norm_for(h, hq)

            if NS == 16:
                # upper P1; first h tile split, wk rides the SP queue
                # behind it, tri/wo deferred past the startup window
                def load_wk():
                    nc.sync.dma_start(out=wk_t[:, 0:NIH, :],
                                      in_=wk_r[:, 0:NIH, :])
                    nc.sync.dma_start(out=wk_t[:, NIH:, :],
                                      in_=wk_r[:, NIH:, :])
                emit_p1(8, split_h=True, after_h=load_wk)
                nc.scalar.dma_start(out=tri_t, in_=tri_d[:])
                for st in range(9, 16):
                    emit_p1(st)
                nc.scalar.dma_start(
                    out=wo_t, in_=wo_d[:].rearrange("(t p) o -> p t o", p=128))
                flush(tq)
                t_avail = set(range(8, 16))

                # pass scheduler state for the hq=3 passes interleaved with
                # the lower P1 stream
                hq3 = []
                for h in range(HL):
                    hq3.append({"h": h, "rem": pass_k_order(3),
                                "started": False})

                def pump(budget):
                    while budget > 0 and hq3:
                        pst = hq3[0]
                        ready = [t for t in pst["rem"] if t in t_avail]
                        if not ready:
                            return budget
                        n = min(budget, len(ready))
                        for t in ready[:n]:
                            is_first = not pst["started"]
                            pst["started"] = True
                            pst["rem"].remove(t)
                            is_last = not pst["rem"]
                            seg_for(pst["h"], t, 3, first=is_first,
                                    last=is_last)
                            if is_last:
                                norm_for(pst["h"], 3)
                                hq3.pop(0)
                        budget -= n
                    return budget

                # lower P1 interleaved with hq=3 segs
                for st in range(7, -1, -1):
                    emit_p1(st)   # flushes T(st+1)
                    t_avail.add(st + 1)
                    pump(6)
                flush(tq)
                t_avail.add(0)
                pump(10 ** 9)

                # remaining passes with phase-3 wedging
                p3q = []
                done_norms = 4  # hq3 norms all done
                order = [(h, hq) for hq in (2, 1, 0) for h in range(HL)]
                for i, (h, hq) in enumerate(order):
                    if hq == 2 and h == 0:
                        p3q.extend(range(12, 16))  # gated by hq3 norms
                    if hq == 1 and h == 0:
                        p3q.extend(range(8, 12))
                    if hq == 0 and h == 0:
                        p3q.extend(range(4, 8))
                    emit_pass(h, hq)
                    if p3q:
                        emit_p3(p3q.pop(0))
                p3q.extend(range(0, 4))
                for st in p3q:
                    emit_p3(st)
            else:
                nc.sync.dma_start(out=wk_t, in_=wk_r)
                nc.scalar.dma_start(out=tri_t, in_=tri_d[:])
                nc.scalar.dma_start(
                    out=wo_t, in_=wo_d[:].rearrange("(t p) o -> p t o", p=128))
                for st in range(NS):
                    emit_p1(st)
                flush(tq)
                for h in range(HL):
                    for hq in range(NQ - 1, -1, -1):
                        emit_pass(h, hq)
                for st in range(NS):
                    emit_p3(st)

    nc.finalize()
    return nc


def rope_tables(S, hd):
    """cos/sin tables matching reference._rope_tables numerics (f32 freqs)."""
    inv = (1.0 / (np.float32(ROPE_BASE) **
                  (np.arange(0, hd, 2, dtype=np.float32) / np.float32(hd))))
    inv = inv.astype(np.float32)
    freqs = (np.arange(S, dtype=np.float32)[:, None] * inv[None, :]
             ).astype(np.float32)
    cos = np.cos(freqs.astype(np.float64)).astype(np.float32)
    sin = np.sin(freqs.astype(np.float64)).astype(np.float32)
    return cos, sin


def make_const_inputs(S):
    """Constant per-core inputs: compact RoPE tables + tri/identity."""
    cos, sin = rope_tables(S, HEAD_DIM)
    bf = ml_dtypes.bfloat16
    return {
        "cs": np.ascontiguousarray(
            np.concatenate([cos, -sin, sin], axis=1).astype(np.float32)),
        "tri": np.triu(np.ones((128, 128), dtype=np.float32)).astype(bf),
        "ident": np.eye(128, dtype=np.float32).astype(bf),
    }


def _is_causal_mask(mask, S):
    m = mask.reshape(S, S)
    rows = np.unique(np.concatenate([np.arange(0, S, S // 64), [S - 1]]))
    for r in rows:
        row = m[r]
        if not np.all(row[:r + 1] == 0.0):
            return False
        if r + 1 < S and not np.all(row[r + 1:] <= -50.0):
            return False
    return True


_NC_CACHE = {}


def kernel(hidden_states, attention_mask, Wqkv, Wo):
    B, S, H = hidden_states.shape
    nh, hd = NUM_HEADS, HEAD_DIM
    HL = nh // (N_CORES // B)       # heads per core
    DL = HL * hd
    G = N_CORES // B                # cores per batch

    if not _is_causal_mask(np.asarray(attention_mask), S):
        return _host_reference(hidden_states, attention_mask, Wqkv, Wo)

    key = (S, H, HL)
    if key not in _NC_CACHE:
        _NC_CACHE[key] = build_nc(S, H, HL)
    nc = _NC_CACHE[key]

    consts = make_const_inputs(S)
    bf = ml_dtypes.bfloat16

    hs = np.asarray(hidden_states, dtype=np.float32)
    Wqkv = np.asarray(Wqkv, dtype=np.float32)
    Wo = np.asarray(Wo, dtype=np.float32)
    hT = [np.ascontiguousarray(hs[b].T) for b in range(B)]

    in_maps = []
    for c in range(N_CORES):
        b, g = divmod(c, G)
        c0 = g * DL
        in_maps.append({
            "hT": hT[b],
            "wq": np.ascontiguousarray(Wqkv[:, c0:c0 + DL]),
            "wk": np.ascontiguousarray(Wqkv[:, H + c0:H + c0 + DL]),
            "wv": np.ascontiguousarray(Wqkv[:, 2 * H + c0:2 * H + c0 + DL]),
            "wo": np.ascontiguousarray(Wo[c0:c0 + DL, :]).astype(bf),
            **consts,
        })

    res = run_bass_kernel_spmd(nc, in_maps, list(range(N_CORES)))
    out = np.empty((B, S, H), dtype=np.float32)
    for b in range(B):
        acc = res.results[b * G]["part"].astype(np.float64)
        for g in range(1, G):
            acc += res.results[b * G + g]["part"].astype(np.float64)
        out[b] = acc.astype(np.float32)
    return out


def _host_reference(hidden_states, attention_mask, Wqkv, Wo):
    """Exact fallback for non-causal masks (numpy, fp32)."""
    B, S, H = hidden_states.shape
    nh, hd = NUM_HEADS, HEAD_DIM
    cos, sin = rope_tables(S, hd)
    qkv = hidden_states.reshape(B * S, H) @ Wqkv
    qkv = qkv.reshape(B, S, 3, nh, hd).transpose(2, 0, 3, 1, 4)
    q, k, v = qkv[0], qkv[1], qkv[2]

    def rope(x):
        x1, x2 = x[..., :hd // 2], x[..., hd // 2:]
        c, s = cos[None, None], sin[None, None]
        return np.concatenate([x1 * c - x2 * s, x2 * c + x1 * s], axis=-1)

    q, k = rope(q), rope(k)
    scores = np.einsum('bhqd,bhkd->bhqk', q, k) * (hd ** -0.5)
    scores = scores + attention_mask.reshape(1, 1, S, S)
    scores -= scores.max(axis=-1, keepdims=True)
    e = np.exp(scores)
    attn = e / e.sum(axis=-1, keepdims=True)
    out = np.einsum('bhqk,bhkd->bhqd', attn, v)
    out = out.transpose(0, 2, 1, 3).reshape(B, S, H)
    return (out @ Wo).astype(np.float32)


# revision 14
# speedup vs baseline: 1.0005x; 1.0005x over previous
"""Causal self-attention (RoPE) Trainium2 Bass kernel, SPMD over 8 NeuronCores.

Sharding: data-parallel over batch (B=2) x tensor-parallel over heads
(16 heads -> 4 heads per core).  core c handles batch c//4, heads
4*(c%4) .. 4*(c%4)+3.  Each core computes its heads' attention output and a
partial out@Wo contribution ([S, H] in f16); the host sums the 4 partials
per batch.

Device pipeline per core (transposed-scores formulation, bf16 attention):
  1. QKV projection (fp32r) from hidden^T; RoPE (f32) on q,k; q,k cast to
     bf16 and PE-transposed (4 tiles into one PSUM bank, single bulk copy);
     v stored natural bf16 with 64 appended ones-columns (V'), so the AV
     matmul replicates the softmax denominator on partitions 64..127 free
     of charge (no partition broadcast needed for normalization).
  2. 512-column q-passes: scores^T[k,q] = K Q^T (bf16) -> exp -> causal tri
     mask on diagonal tiles -> O'^T[0:128, q] = V'^T P~^T accumulated in a
     single PSUM bank per pass; normalization = reciprocal of rows 64:128
     times rows 0:64 (two DVE ops, PSUM-direct).
  3. O^T normalized into on_t (bf16) -> partial = O_norm @ Wo (bf16) in
     512-wide chunks -> f16 partial out, DMA'd on the DVE queue.
"""

import sys
import numpy as np

for _p in ("/opt/trn_rl_repo", "/root/.axon_site/_ro/trn_rl_repo"):
    if _p not in sys.path:
        sys.path.append(_p)

import ml_dtypes
import concourse.bacc as bacc
from concourse import mybir
from concourse.tile import TileContext
from concourse.bass_utils import run_bass_kernel_spmd

F32 = mybir.dt.float32
F32R = mybir.dt.float32r
BF16 = mybir.dt.bfloat16
F16 = mybir.dt.float16
EXP = mybir.ActivationFunctionType.Exp

NUM_HEADS = 16
HEAD_DIM = 64
ROPE_BASE = 160000.0
N_CORES = 8


def build_nc(S, H, HL, debug=False):
    """Build the SPMD Bass program.

    S: sequence length; H: hidden size; HL: heads per core (local).
    """
    DL = HL * HEAD_DIM          # local channels (256)
    NI = H // 128               # contraction tiles for projections (8)
    NS = S // 128               # sequence tiles (16)
    CT = max(DL // 128, 1)      # channel tiles (2)
    NQ = S // 512               # 512-wide q passes per head (4)
    scale = HEAD_DIM ** -0.5
    assert DL % 128 == 0 and H % 128 == 0 and S % 1024 == 0

    nc = bacc.Bacc("TRN2", target_bir_lowering=False, debug=False,
                   num_devices=N_CORES)
    hT_d = nc.declare_dram_parameter("hT", [H, S], F32, isOutput=False)
    wq_d = nc.declare_dram_parameter("wq", [H, DL], F32, isOutput=False)
    wk_d = nc.declare_dram_parameter("wk", [H, DL], F32, isOutput=False)
    wv_d = nc.declare_dram_parameter("wv", [H, DL], F32, isOutput=False)
    wo_d = nc.declare_dram_parameter("wo", [DL, H], BF16, isOutput=False)
    cs_d = nc.declare_dram_parameter("cs", [S, 96], F32, isOutput=False)
    tri_d = nc.declare_dram_parameter("tri", [128, 128], BF16, isOutput=False)
    id_d = nc.declare_dram_parameter("ident", [128, 128], BF16, isOutput=False)
    out_d = nc.declare_dram_parameter("part", [S, H], F16, isOutput=True)

    with TileContext(nc) as tc:
        with (
            tc.tile_pool(name="w", bufs=1) as w_pool,
            tc.tile_pool(name="persist", bufs=1) as pers,
            tc.tile_pool(name="hstream", bufs=3) as hs_pool,
            tc.tile_pool(name="xall", bufs=3) as xa_pool,
            tc.tile_pool(name="rope", bufs=3) as rope_pool,
            tc.tile_pool(name="qksb", bufs=3) as qk_pool,
            tc.tile_pool(name="psb", bufs=6) as p_pool,
            tc.tile_pool(name="norm", bufs=3) as n_pool,
            tc.tile_pool(name="osb", bufs=3) as o_pool,
            tc.tile_pool(name="ps_sc", bufs=3, space="PSUM") as ps_sc,
            tc.tile_pool(name="ps_o", bufs=2, space="PSUM") as ps_o,
            tc.tile_pool(name="ps_x", bufs=3, space="PSUM") as ps_x,
        ):
            # --- weights / constants (resident) ---
            wq_t = w_pool.tile([128, NI, DL], F32R)
            wk_t = w_pool.tile([128, NI, DL], F32R)
            wv_t = w_pool.tile([128, NI, DL], F32R)
            wo_t = w_pool.tile([128, CT, H], BF16)
            cs_t = w_pool.tile([128, NS, 96], F32)
            tri_t = w_pool.tile([128, 128], BF16)
            id_t = w_pool.tile([128, 128], BF16)

            NIH = NI // 2
            # weight streams split across both HWDGE queues so the first
            # projection groups aren't gated on one serialized queue:
            # ACT carries wq/wv (q and v groups run first), SP carries wk
            # (behind the first h tile).  tri/wo are deferred below.
            wq_r = wq_d[:].rearrange("(t p) d -> p t d", p=128).bitcast(F32R)
            wv_r = wv_d[:].rearrange("(t p) d -> p t d", p=128).bitcast(F32R)
            wk_r = wk_d[:].rearrange("(t p) d -> p t d", p=128).bitcast(F32R)
            nc.scalar.dma_start(out=wq_t[:, 0:NIH, :], in_=wq_r[:, 0:NIH, :])
            nc.scalar.dma_start(out=wq_t[:, NIH:, :], in_=wq_r[:, NIH:, :])
            nc.scalar.dma_start(out=wv_t[:, 0:NIH, :], in_=wv_r[:, 0:NIH, :])
            nc.scalar.dma_start(out=wv_t[:, NIH:, :], in_=wv_r[:, NIH:, :])
            nc.scalar.dma_start(
                out=cs_t, in_=cs_d[:].rearrange("(t p) d -> p t d", p=128))
            nc.scalar.dma_start(out=id_t, in_=id_d[:])

            # persistent activations
            qkT = pers.tile([128, 2, CT, S], BF16)   # [d, q/k, ct, s]
            vv = pers.tile([128, NS, HL, 128], BF16)  # v cols 0:64, ones 64:128
            on_t = pers.tile([128, CT, S], BF16)
            nc.gpsimd.memset(vv[:, :, :, 64:128], 1.0)

            hT_r = hT_d[:].rearrange("(t p) s -> p t s", p=128).bitcast(F32R)

            # ---------------- deferred-emission queues ----------------
            tq = []   # pending transpose closures (phase 1)
            avq = []  # pending AV closures (phase 2)

            def flush(qu, keep=0):
                while len(qu) > keep:
                    qu.pop(0)()

            # ---------------- phase 2 ----------------
            o_tiles = {}

            def seg_for(h, t, hq, first=False, last=False):
                """scores+exp+mask for k-tile t against q in
                [max(128t, 512*hq), 512*(hq+1)); AV deferred."""
                sg = hq * 512
                k0 = t * 128
                o0 = max(k0 - sg, 0)
                if first:
                    assert o0 == 0, "first seg of a pass must cover the q-range"
                base = (h % 2) * 64
                ct = h // 2
                kT = qkT[base:base + 64, 1, ct, k0:k0 + 128]
                qT = qkT[base:base + 64, 0, ct, sg + o0:sg + 512]
                if (h, hq) not in o_tiles:
                    o_tiles[(h, hq)] = ps_o.tile([128, 512], F32, tag="o",
                                                 name=f"o_{h}_{hq}")
                o_ps = o_tiles[(h, hq)]
                sc = ps_sc.tile([128, 512], F32, tag="sc")
                nc.tensor.matmul(sc[:, o0:512], kT, qT, start=True, stop=True)
                flush(avq, keep=3)
                p = p_pool.tile([128, 512], BF16, tag="p")
                nc.scalar.activation(p[:, o0:512], sc[:, o0:512], EXP,
                                     scale=scale)
                if k0 >= sg:  # diagonal tile: causal mask
                    nc.vector.tensor_mul(p[:, o0:o0 + 128], p[:, o0:o0 + 128],
                                         tri_t)

                def av():
                    nc.tensor.matmul(o_ps[:, o0:512], vv[:, t, h, :],
                                     p[:, o0:512], start=first, stop=last,
                                     skip_group_check=True)
                avq.append(av)

            def norm_for(h, hq):
                flush(avq)
                sg = hq * 512
                base = (h % 2) * 64
                ct = h // 2
                o_ps = o_tiles.pop((h, hq))
                # rows 64:128 of O' hold the softmax denominator replicated
                # (ones-columns of V'); normalize PSUM-direct: two DVE ops
                r_sb = n_pool.tile([64, 512], F32, tag="r")
                nc.vector.reciprocal(r_sb, o_ps[64:128, :])
                nc.vector.tensor_mul(on_t[base:base + 64, ct, sg:sg + 512],
                                     o_ps[0:64, :], r_sb)

            # ---------------- phase 1 ----------------
            def emit_p1(st, split_h=False, after_h=None):
                s0 = st * 128
                h_t = hs_pool.tile([128, NI, 128], F32R, tag="h")
                if split_h:
                    nc.sync.dma_start(out=h_t[:, 0:NIH, :],
                                      in_=hT_r[:, 0:NIH, s0:s0 + 128])
                    nc.sync.dma_start(out=h_t[:, NIH:, :],
                                      in_=hT_r[:, NIH:, s0:s0 + 128])
                else:
                    nc.sync.dma_start(out=h_t, in_=hT_r[:, :, s0:s0 + 128])
                if after_h is not None:
                    after_h()
                qk_ps = ps_x.tile([128, 512], F32, tag="px")
                v_ps = ps_x.tile([128, 512], F32, tag="px")
                for w_t, ps, col in ((wq_t, qk_ps, 0), (wv_t, v_ps, 0),
                                     (wk_t, qk_ps, 256)):
                    for i in range(NI):
                        nc.tensor.matmul(ps[:, col:col + DL], h_t[:, i, :],
                                         w_t[:, i, :],
                                         start=(i == 0), stop=(i == NI - 1))
                flush(tq)

                # drain on DVE+Pool only: the ACT queue stays exp-only, so
                # phase-1 never stalls behind weight-DMA issues or exps
                x_all = xa_pool.tile([128, 512], F32, tag="xa")
                nc.vector.tensor_copy(x_all, qk_ps)
                nc.gpsimd.tensor_copy(
                    vv[:, st, :, 0:64],
                    v_ps[:, 0:DL].rearrange("p (h d) -> p h d", d=64))

                cosb = cs_t[:, st, 0:32].unsqueeze(1).broadcast_to(
                    [128, 2 * HL, 32])
                sinmb = cs_t[:, st, 32:64].unsqueeze(1).broadcast_to(
                    [128, HL, 32])
                sinpb = cs_t[:, st, 64:96].unsqueeze(1).broadcast_to(
                    [128, HL, 32])
                x_sb = {}
                for qk in (0, 1):
                    xsrc = x_all[:, qk * DL:(qk + 1) * DL]
                    x4 = xsrc.rearrange("p (h two d) -> p h two d", two=2,
                                        d=32)
                    a_t = rope_pool.tile([128, DL], F32, tag="ra")
                    nc.vector.tensor_mul(
                        a_t.rearrange("p (r d) -> p r d", d=32),
                        xsrc.rearrange("p (r d) -> p r d", d=32), cosb)
                    b_t = rope_pool.tile([128, DL], F32, tag="rb")
                    b4 = b_t.rearrange("p (h two d) -> p h two d", two=2,
                                       d=32)
                    nc.vector.tensor_mul(b4[:, :, 0, :], x4[:, :, 1, :],
                                         sinmb)
                    nc.vector.tensor_mul(b4[:, :, 1, :], x4[:, :, 0, :],
                                         sinpb)
                    xs = qk_pool.tile([128, DL], BF16, tag=f"x{qk}")
                    nc.gpsimd.tensor_add(xs, a_t, b_t)
                    x_sb[qk] = xs

                def transposes():
                    t_ps = ps_x.tile([128, 1024], BF16, tag="px")
                    for qk in (0, 1):
                        for ctp in range(CT):
                            idx = qk * CT + ctp
                            nc.tensor.transpose(
                                t_ps[:, idx * 128:(idx + 1) * 128],
                                x_sb[qk][:, ctp * 128:(ctp + 1) * 128],
                                id_t)
                    nc.vector.tensor_copy(
                        qkT[:, :, :, s0:s0 + 128],
                        t_ps[:, 0:512].rearrange("p (a b s) -> p a b s",
                                                 a=2, b=CT))
                tq.append(transposes)

            # ---------------- phase 3 ----------------
            def emit_p3(st):
                s0 = st * 128
                out_sb = o_pool.tile([128, H], F16, tag="out")
                for oc, ceng in ((0, nc.gpsimd), (512, nc.vector)):
                    ps = ps_x.tile([128, 512], F32, tag="px")
                    for ctp in range(CT):
                        nc.tensor.matmul(ps, on_t[:, ctp, s0:s0 + 128],
                                         wo_t[:, ctp, oc:oc + 512],
                                         start=(ctp == 0),
                                         stop=(ctp == CT - 1))
                    ceng.tensor_copy(out_sb[:, oc:oc + 512], ps)
                nc.sync.dma_start(out=out_d[s0:s0 + 128, :], in_=out_sb)

            # ---------------- orchestration ----------------
            def pass_k_order(hq):
                first = 4 * hq
                return ([first] + list(range(first + 1, 4 * (hq + 1))) +
                        list(range(first - 1, -1, -1)))

            def emit_pass(h, hq):
                ts = pass_k_order(hq)
                for j, t in enumerate(ts):
                    seg_for(h, t, hq, first=(j == 0), last=(j == len(ts) - 1))
                norm_for(h, hq)

            if NS == 16:
                # global pass scheduler: 16 passes processed in order, segs
                # emitted as their k-tiles' transposes land; a pass may open
                # on ANY k-tile with t <= 4*hq (full q coverage).  P3 tiles
                # unlock when their q-tier's norms are all emitted and get
                # wedged one per norm to fill pass-transition bubbles.
                t_avail = set()
                passes = [{"h": h, "hq": hq, "rem": pass_k_order(hq),
                           "started": False}
                          for hq in (3, 2, 1, 0) for h in range(HL)]
                tier_left = {hq: HL for hq in range(NQ)}
                p3q = []

                def pump(budget):
                    while budget > 0 and passes:
                        pst = passes[0]
                        hq = pst["hq"]
                        if pst["started"]:
                            ready = [t for t in pst["rem"] if t in t_avail]
                        else:
                            ready = [t for t in pst["rem"]
                                     if t in t_avail and t <= 4 * hq]
                        if not ready:
                            return budget
                        n = min(budget, len(ready))
                        for t in ready[:n]:
                            is_first = not pst["started"]
                            pst["started"] = True
                            pst["rem"].remove(t)
                            is_last = not pst["rem"]
                            seg_for(pst["h"], t, hq, first=is_first,
                                    last=is_last)
                            if is_last:
                                norm_for(pst["h"], hq)
                                passes.pop(0)
                                tier_left[hq] -= 1
                                if tier_left[hq] == 0:
                                    p3q.extend(range(4 * hq, 4 * hq + 4))
                                if p3q:
                                    emit_p3(p3q.pop(0))
                        budget -= n
                    return budget

                # upper P1; first h tile split, wk rides the SP queue
                # behind it, tri/wo deferred past the startup window
                def load_wk():
                    nc.sync.dma_start(out=wk_t[:, 0:NIH, :],
                                      in_=wk_r[:, 0:NIH, :])
                    nc.sync.dma_start(out=wk_t[:, NIH:, :],
                                      in_=wk_r[:, NIH:, :])
                emit_p1(8, split_h=True, after_h=load_wk)
                nc.scalar.dma_start(out=tri_t, in_=tri_d[:])
                for st in range(9, 16):
                    emit_p1(st)   # flushes T(st-1)
                    t_avail.add(st - 1)
                    pump(3)
                nc.scalar.dma_start(
                    out=wo_t, in_=wo_d[:].rearrange("(t p) o -> p t o", p=128))
                for st in range(7, -1, -1):
                    emit_p1(st)
                    t_avail.add(st + 1 if st < 7 else 15)
                    pump(6)
                flush(tq)
                t_avail.update(range(NS))
                pump(10 ** 9)
                for st in p3q:
                    emit_p3(st)
            else:
                nc.sync.dma_start(out=wk_t, in_=wk_r)
                nc.scalar.dma_start(out=tri_t, in_=tri_d[:])
                nc.scalar.dma_start(
                    out=wo_t, in_=wo_d[:].rearrange("(t p) o -> p t o", p=128))
                for st in range(NS):
                    emit_p1(st)
                flush(tq)
                for h in range(HL):
                    for hq in range(NQ - 1, -1, -1):
                        emit_pass(h, hq)
                for st in range(NS):
                    emit_p3(st)

    nc.finalize()
    return nc


def rope_tables(S, hd):
    """cos/sin tables matching reference._rope_tables numerics (f32 freqs)."""
    inv = (1.0 / (np.float32(ROPE_BASE) **
                  (np.arange(0, hd, 2, dtype=np.float32) / np.float32(hd))))
    inv = inv.astype(np.float32)
    freqs = (np.arange(S, dtype=np.float32)[:, None] * inv[None, :]
             ).astype(np.float32)
    cos = np.cos(freqs.astype(np.float64)).astype(np.float32)
    sin = np.sin(freqs.astype(np.float64)).astype(np.float32)
    return cos, sin


def make_const_inputs(S):
    """Constant per-core inputs: compact RoPE tables + tri/identity."""
    cos, sin = rope_tables(S, HEAD_DIM)
    bf = ml_dtypes.bfloat16
    return {
        "cs": np.ascontiguousarray(
            np.concatenate([cos, -sin, sin], axis=1).astype(np.float32)),
        "tri": np.triu(np.ones((128, 128), dtype=np.float32)).astype(bf),
        "ident": np.eye(128, dtype=np.float32).astype(bf),
    }


def _is_causal_mask(mask, S):
    m = mask.reshape(S, S)
    rows = np.unique(np.concatenate([np.arange(0, S, S // 64), [S - 1]]))
    for r in rows:
        row = m[r]
        if not np.all(row[:r + 1] == 0.0):
            return False
        if r + 1 < S and not np.all(row[r + 1:] <= -50.0):
            return False
    return True


_NC_CACHE = {}


def kernel(hidden_states, attention_mask, Wqkv, Wo):
    B, S, H = hidden_states.shape
    nh, hd = NUM_HEADS, HEAD_DIM
    HL = nh // (N_CORES // B)       # heads per core
    DL = HL * hd
    G = N_CORES // B                # cores per batch

    if not _is_causal_mask(np.asarray(attention_mask), S):
        return _host_reference(hidden_states, attention_mask, Wqkv, Wo)

    key = (S, H, HL)
    if key not in _NC_CACHE:
        _NC_CACHE[key] = build_nc(S, H, HL)
    nc = _NC_CACHE[key]

    consts = make_const_inputs(S)
    bf = ml_dtypes.bfloat16

    hs = np.asarray(hidden_states, dtype=np.float32)
    Wqkv = np.asarray(Wqkv, dtype=np.float32)
    Wo = np.asarray(Wo, dtype=np.float32)
    hT = [np.ascontiguousarray(hs[b].T) for b in range(B)]

    in_maps = []
    for c in range(N_CORES):
        b, g = divmod(c, G)
        c0 = g * DL
        in_maps.append({
            "hT": hT[b],
            "wq": np.ascontiguousarray(Wqkv[:, c0:c0 + DL]),
            "wk": np.ascontiguousarray(Wqkv[:, H + c0:H + c0 + DL]),
            "wv": np.ascontiguousarray(Wqkv[:, 2 * H + c0:2 * H + c0 + DL]),
            "wo": np.ascontiguousarray(Wo[c0:c0 + DL, :]).astype(bf),
            **consts,
        })

    res = run_bass_kernel_spmd(nc, in_maps, list(range(N_CORES)))
    out = np.empty((B, S, H), dtype=np.float32)
    for b in range(B):
        acc = res.results[b * G]["part"].astype(np.float64)
        for g in range(1, G):
            acc += res.results[b * G + g]["part"].astype(np.float64)
        out[b] = acc.astype(np.float32)
    return out


def _host_reference(hidden_states, attention_mask, Wqkv, Wo):
    """Exact fallback for non-causal masks (numpy, fp32)."""
    B, S, H = hidden_states.shape
    nh, hd = NUM_HEADS, HEAD_DIM
    cos, sin = rope_tables(S, hd)
    qkv = hidden_states.reshape(B * S, H) @ Wqkv
    qkv = qkv.reshape(B, S, 3, nh, hd).transpose(2, 0, 3, 1, 4)
    q, k, v = qkv[0], qkv[1], qkv[2]

    def rope(x):
        x1, x2 = x[..., :hd // 2], x[..., hd // 2:]
        c, s = cos[None, None], sin[None, None]
        return np.concatenate([x1 * c - x2 * s, x2 * c + x1 * s], axis=-1)

    q, k = rope(q), rope(k)
    scores = np.einsum('bhqd,bhkd->bhqk', q, k) * (hd ** -0.5)
    scores = scores + attention_mask.reshape(1, 1, S, S)
    scores -= scores.max(axis=-1, keepdims=True)
    e = np.exp(scores)
    attn = e / e.sum(axis=-1, keepdims=True)
    out = np.einsum('bhqk,bhkd->bhqd', attn, v)
    out = out.transpose(0, 2, 1, 3).reshape(B, S, H)
    return (out @ Wo).astype(np.float32)


# revision 19
# speedup vs baseline: 1.0265x; 1.0260x over previous
"""Causal self-attention (RoPE) Trainium2 Bass kernel, SPMD over 8 NeuronCores.

Sharding: data-parallel over batch (B=2) x tensor-parallel over heads
(16 heads -> 4 heads per core).  core c handles batch c//4, heads
4*(c%4) .. 4*(c%4)+3.  Each core computes its heads' attention output and a
partial out@Wo contribution ([S, H] in f16); the host sums the 4 partials
per batch.

Device pipeline per core (transposed-scores formulation, bf16 attention):
  1. QKV projection (fp32r) from hidden^T; RoPE (f32) on q,k; q,k cast to
     bf16 and PE-transposed (4 tiles into one PSUM bank, single bulk copy);
     v stored natural bf16 with 64 appended ones-columns (V'), so the AV
     matmul replicates the softmax denominator on partitions 64..127 free
     of charge (no partition broadcast needed for normalization).
  2. 512-column q-passes: scores^T[k,q] = K Q^T (bf16) -> exp -> causal tri
     mask on diagonal tiles -> O'^T[0:128, q] = V'^T P~^T accumulated in a
     single PSUM bank per pass; normalization = reciprocal of rows 64:128
     times rows 0:64 (two DVE ops, PSUM-direct).
  3. O^T normalized into on_t (bf16) -> partial = O_norm @ Wo (bf16) in
     512-wide chunks -> f16 partial out, DMA'd on the DVE queue.
"""

import sys
import numpy as np

for _p in ("/opt/trn_rl_repo", "/root/.axon_site/_ro/trn_rl_repo"):
    if _p not in sys.path:
        sys.path.append(_p)

import ml_dtypes
import concourse.bacc as bacc
from concourse import mybir
from concourse.tile import TileContext
from concourse.bass_utils import run_bass_kernel_spmd

F32 = mybir.dt.float32
F32R = mybir.dt.float32r
BF16 = mybir.dt.bfloat16
F16 = mybir.dt.float16
EXP = mybir.ActivationFunctionType.Exp

NUM_HEADS = 16
HEAD_DIM = 64
ROPE_BASE = 160000.0
N_CORES = 8


def build_nc(S, H, HL, debug=False):
    """Build the SPMD Bass program.

    S: sequence length; H: hidden size; HL: heads per core (local).
    """
    DL = HL * HEAD_DIM          # local channels (256)
    NI = H // 128               # contraction tiles for projections (8)
    NS = S // 128               # sequence tiles (16)
    CT = max(DL // 128, 1)      # channel tiles (2)
    NQ = S // 512               # 512-wide q passes per head (4)
    scale = HEAD_DIM ** -0.5
    assert DL % 128 == 0 and H % 128 == 0 and S % 1024 == 0

    nc = bacc.Bacc("TRN2", target_bir_lowering=False, debug=False,
                   num_devices=N_CORES)
    hT_d = nc.declare_dram_parameter("hT", [H, S], F32, isOutput=False)
    wq_d = nc.declare_dram_parameter("wq", [H, DL], F32, isOutput=False)
    wk_d = nc.declare_dram_parameter("wk", [H, DL], F32, isOutput=False)
    wv_d = nc.declare_dram_parameter("wv", [H, DL], F32, isOutput=False)
    wo_d = nc.declare_dram_parameter("wo", [DL, H], BF16, isOutput=False)
    cs_d = nc.declare_dram_parameter("cs", [S, 96], F32, isOutput=False)
    tri_d = nc.declare_dram_parameter("tri", [128, 128], BF16, isOutput=False)
    id_d = nc.declare_dram_parameter("ident", [128, 128], BF16, isOutput=False)
    out_d = nc.declare_dram_parameter("part", [S, H], F16, isOutput=True)

    with TileContext(nc) as tc:
        with (
            tc.tile_pool(name="w", bufs=1) as w_pool,
            tc.tile_pool(name="persist", bufs=1) as pers,
            tc.tile_pool(name="hstream", bufs=3) as hs_pool,
            tc.tile_pool(name="xall", bufs=3) as xa_pool,
            tc.tile_pool(name="rope", bufs=3) as rope_pool,
            tc.tile_pool(name="qksb", bufs=3) as qk_pool,
            tc.tile_pool(name="psb", bufs=6) as p_pool,
            tc.tile_pool(name="norm", bufs=3) as n_pool,
            tc.tile_pool(name="osb", bufs=3) as o_pool,
            tc.tile_pool(name="ps_sc", bufs=3, space="PSUM") as ps_sc,
            tc.tile_pool(name="ps_o", bufs=1, space="PSUM") as ps_o,
            tc.tile_pool(name="ps_x", bufs=3, space="PSUM") as ps_x,
        ):
            # --- weights / constants (resident) ---
            wq_t = w_pool.tile([128, NI, DL], F32R)
            wk_t = w_pool.tile([128, NI, DL], F32R)
            wv_t = w_pool.tile([128, NI, DL], F32R)
            wo_t = w_pool.tile([128, CT, H], BF16)
            cs_t = w_pool.tile([128, NS, 96], F32)
            tri_t = w_pool.tile([128, 128], BF16)
            id_t = w_pool.tile([128, 128], BF16)

            NIH = NI // 2
            # weight streams split across both HWDGE queues so the first
            # projection groups aren't gated on one serialized queue:
            # ACT carries wq/wv (q and v groups run first), SP carries wk
            # (behind the first h tile).  tri/wo are deferred below.
            wq_r = wq_d[:].rearrange("(t p) d -> p t d", p=128).bitcast(F32R)
            wv_r = wv_d[:].rearrange("(t p) d -> p t d", p=128).bitcast(F32R)
            wk_r = wk_d[:].rearrange("(t p) d -> p t d", p=128).bitcast(F32R)
            nc.scalar.dma_start(out=wq_t[:, 0:NIH, :], in_=wq_r[:, 0:NIH, :])
            nc.scalar.dma_start(out=wq_t[:, NIH:, :], in_=wq_r[:, NIH:, :])
            nc.scalar.dma_start(out=wv_t[:, 0:NIH, :], in_=wv_r[:, 0:NIH, :])
            nc.scalar.dma_start(out=wv_t[:, NIH:, :], in_=wv_r[:, NIH:, :])
            nc.scalar.dma_start(
                out=cs_t, in_=cs_d[:].rearrange("(t p) d -> p t d", p=128))
            nc.scalar.dma_start(out=id_t, in_=id_d[:])

            # persistent activations
            qkT = pers.tile([128, 2, CT, S], BF16)   # [d, q/k, ct, s]
            vv = pers.tile([128, NS, HL, 128], BF16)  # v cols 0:64, ones 64:128
            on_t = pers.tile([128, CT, S], BF16)
            nc.gpsimd.memset(vv[:, :, :, 64:128], 1.0)

            hT_r = hT_d[:].rearrange("(t p) s -> p t s", p=128).bitcast(F32R)

            # ---------------- deferred-emission queues ----------------
            tq = []   # pending transpose closures (phase 1)
            avq = []  # pending AV closures (phase 2)

            def flush(qu, keep=0):
                while len(qu) > keep:
                    qu.pop(0)()

            # ---------------- phase 2 ----------------
            # two PSUM o-accumulator slots, handed out as free tags so a
            # pass always reuses the slot of the LAST pass to finish (a
            # fixed ring would reallocate a still-accumulating slot)
            o_tiles = {}
            free_otags = ["oA", "oB"]

            def seg_for(h, t, hq, first=False, last=False):
                """scores+exp+mask for k-tile t against q in
                [max(128t, 512*hq), 512*(hq+1)); AV deferred."""
                sg = hq * 512
                k0 = t * 128
                o0 = max(k0 - sg, 0)
                if first:
                    assert o0 == 0, "first seg of a pass must cover the q-range"
                base = (h % 2) * 64
                ct = h // 2
                kT = qkT[base:base + 64, 1, ct, k0:k0 + 128]
                qT = qkT[base:base + 64, 0, ct, sg + o0:sg + 512]
                if (h, hq) not in o_tiles:
                    otag = free_otags.pop(0)
                    o_tiles[(h, hq)] = (
                        ps_o.tile([128, 512], F32, tag=otag,
                                  name=f"o_{h}_{hq}"), otag)
                o_ps = o_tiles[(h, hq)][0]
                sc = ps_sc.tile([128, 512], F32, tag="sc")
                nc.tensor.matmul(sc[:, o0:512], kT, qT, start=True, stop=True)
                flush(avq, keep=3)
                p = p_pool.tile([128, 512], BF16, tag="p")
                nc.scalar.activation(p[:, o0:512], sc[:, o0:512], EXP,
                                     scale=scale)
                if k0 >= sg:  # diagonal tile: causal mask
                    nc.vector.tensor_mul(p[:, o0:o0 + 128], p[:, o0:o0 + 128],
                                         tri_t)

                def av():
                    nc.tensor.matmul(o_ps[:, o0:512], vv[:, t, h, :],
                                     p[:, o0:512], start=first, stop=last,
                                     skip_group_check=True)
                avq.append(av)

            def norm_for(h, hq):
                flush(avq)
                sg = hq * 512
                base = (h % 2) * 64
                ct = h // 2
                o_ps, otag = o_tiles.pop((h, hq))
                free_otags.append(otag)
                # rows 64:128 of O' hold the softmax denominator replicated
                # (ones-columns of V'); normalize PSUM-direct: two DVE ops
                r_sb = n_pool.tile([64, 512], F32, tag="r")
                nc.vector.reciprocal(r_sb, o_ps[64:128, :])
                nc.vector.tensor_mul(on_t[base:base + 64, ct, sg:sg + 512],
                                     o_ps[0:64, :], r_sb)

            # ---------------- phase 1 ----------------
            def emit_p1(st, split_h=False, after_h=None):
                s0 = st * 128
                h_t = hs_pool.tile([128, NI, 128], F32R, tag="h")
                if split_h:
                    nc.sync.dma_start(out=h_t[:, 0:NIH, :],
                                      in_=hT_r[:, 0:NIH, s0:s0 + 128])
                    nc.sync.dma_start(out=h_t[:, NIH:, :],
                                      in_=hT_r[:, NIH:, s0:s0 + 128])
                else:
                    nc.sync.dma_start(out=h_t, in_=hT_r[:, :, s0:s0 + 128])
                if after_h is not None:
                    after_h()
                qk_ps = ps_x.tile([128, 512], F32, tag="px")
                v_ps = ps_x.tile([128, 512], F32, tag="px")
                for w_t, ps, col in ((wq_t, qk_ps, 0), (wv_t, v_ps, 0),
                                     (wk_t, qk_ps, 256)):
                    for i in range(NI):
                        nc.tensor.matmul(ps[:, col:col + DL], h_t[:, i, :],
                                         w_t[:, i, :],
                                         start=(i == 0), stop=(i == NI - 1))
                flush(tq)

                # drain on DVE+Pool only: the ACT queue stays exp-only, so
                # phase-1 never stalls behind weight-DMA issues or exps
                x_all = xa_pool.tile([128, 512], F32, tag="xa")
                nc.vector.tensor_copy(x_all, qk_ps)
                nc.gpsimd.tensor_copy(
                    vv[:, st, :, 0:64],
                    v_ps[:, 0:DL].rearrange("p (h d) -> p h d", d=64))

                cosb = cs_t[:, st, 0:32].unsqueeze(1).broadcast_to(
                    [128, 2 * HL, 32])
                sinmb = cs_t[:, st, 32:64].unsqueeze(1).broadcast_to(
                    [128, HL, 32])
                sinpb = cs_t[:, st, 64:96].unsqueeze(1).broadcast_to(
                    [128, HL, 32])
                x_sb = {}
                for qk in (0, 1):
                    xsrc = x_all[:, qk * DL:(qk + 1) * DL]
                    x4 = xsrc.rearrange("p (h two d) -> p h two d", two=2,
                                        d=32)
                    a_t = rope_pool.tile([128, DL], F32, tag="ra")
                    nc.vector.tensor_mul(
                        a_t.rearrange("p (r d) -> p r d", d=32),
                        xsrc.rearrange("p (r d) -> p r d", d=32), cosb)
                    b_t = rope_pool.tile([128, DL], F32, tag="rb")
                    b4 = b_t.rearrange("p (h two d) -> p h two d", two=2,
                                       d=32)
                    nc.vector.tensor_mul(b4[:, :, 0, :], x4[:, :, 1, :],
                                         sinmb)
                    nc.vector.tensor_mul(b4[:, :, 1, :], x4[:, :, 0, :],
                                         sinpb)
                    xs = qk_pool.tile([128, DL], BF16, tag=f"x{qk}")
                    nc.gpsimd.tensor_add(xs, a_t, b_t)
                    x_sb[qk] = xs

                def transposes():
                    t_ps = ps_x.tile([128, 1024], BF16, tag="px")
                    for qk in (0, 1):
                        for ctp in range(CT):
                            idx = qk * CT + ctp
                            nc.tensor.transpose(
                                t_ps[:, idx * 128:(idx + 1) * 128],
                                x_sb[qk][:, ctp * 128:(ctp + 1) * 128],
                                id_t)
                    nc.vector.tensor_copy(
                        qkT[:, :, :, s0:s0 + 128],
                        t_ps[:, 0:512].rearrange("p (a b s) -> p a b s",
                                                 a=2, b=CT))
                tq.append(transposes)

            # ---------------- phase 3 ----------------
            def emit_p3(st):
                s0 = st * 128
                out_sb = o_pool.tile([128, H], F16, tag="out")
                for oc, ceng in ((0, nc.gpsimd), (512, nc.vector)):
                    ps = ps_x.tile([128, 512], F32, tag="px")
                    for ctp in range(CT):
                        nc.tensor.matmul(ps, on_t[:, ctp, s0:s0 + 128],
                                         wo_t[:, ctp, oc:oc + 512],
                                         start=(ctp == 0),
                                         stop=(ctp == CT - 1))
                    ceng.tensor_copy(out_sb[:, oc:oc + 512], ps)
                nc.sync.dma_start(out=out_d[s0:s0 + 128, :], in_=out_sb)

            # ---------------- orchestration ----------------
            def pass_k_order(hq):
                first = 4 * hq
                return ([first] + list(range(first + 1, 4 * (hq + 1))) +
                        list(range(first - 1, -1, -1)))

            def emit_pass(h, hq):
                ts = pass_k_order(hq)
                for j, t in enumerate(ts):
                    seg_for(h, t, hq, first=(j == 0), last=(j == len(ts) - 1))
                norm_for(h, hq)

            if NS == 16:
                # global pass scheduler: 16 passes processed in order, segs
                # emitted as their k-tiles' transposes land; a pass may open
                # on ANY k-tile with t <= 4*hq (full q coverage).  P3 tiles
                # unlock when their q-tier's norms are all emitted and get
                # wedged one per norm to fill pass-transition bubbles.
                t_avail = set()
                passes = [{"h": h, "hq": hq, "rem": pass_k_order(hq),
                           "started": False}
                          for hq in (3, 2, 1, 0) for h in range(HL)]
                tier_left = {hq: HL for hq in range(NQ)}
                p3q = []

                def pump_one(pst, budget):
                    """Emit up to budget ready segs from one pass; returns
                    (emitted, finished)."""
                    hq = pst["hq"]
                    if pst["started"]:
                        ready = [t for t in pst["rem"] if t in t_avail]
                    elif not free_otags:
                        return 0, False   # no accumulator slot free yet
                    else:
                        ready = [t for t in pst["rem"]
                                 if t in t_avail and t <= 4 * hq]
                    n = min(budget, len(ready))
                    fin = False
                    for t in ready[:n]:
                        is_first = not pst["started"]
                        pst["started"] = True
                        pst["rem"].remove(t)
                        is_last = not pst["rem"]
                        seg_for(pst["h"], t, hq, first=is_first, last=is_last)
                        if is_last:
                            norm_for(pst["h"], hq)
                            tier_left[hq] -= 1
                            if tier_left[hq] == 0:
                                p3q.extend(range(4 * hq, 4 * hq + 4))
                            if p3q:
                                emit_p3(p3q.pop(0))
                            fin = True
                    return n, fin

                def pump(budget):
                    # two o-accumulator slots -> at most two passes in
                    # flight; drain the front pass first
                    while budget > 0 and passes:
                        n0, fin = pump_one(passes[0], budget)
                        if fin:
                            passes.pop(0)
                            budget -= n0
                            continue
                        budget -= n0
                        if budget <= 0 or len(passes) < 2:
                            return budget
                        n1, fin1 = pump_one(passes[1], budget)
                        if fin1:
                            passes.pop(1)
                        budget -= n1
                        if n0 == 0 and n1 == 0:
                            return budget
                    return budget

                # upper P1; first h tile split, wk rides the SP queue
                # behind it, tri/wo deferred past the startup window
                def load_wk():
                    nc.sync.dma_start(out=wk_t[:, 0:NIH, :],
                                      in_=wk_r[:, 0:NIH, :])
                    nc.sync.dma_start(out=wk_t[:, NIH:, :],
                                      in_=wk_r[:, NIH:, :])
                emit_p1(8, split_h=True, after_h=load_wk)
                nc.scalar.dma_start(out=tri_t, in_=tri_d[:])
                for st in range(9, 16):
                    emit_p1(st)   # flushes T(st-1)
                    t_avail.add(st - 1)
                    pump(3)
                nc.scalar.dma_start(
                    out=wo_t, in_=wo_d[:].rearrange("(t p) o -> p t o", p=128))
                for st in range(7, -1, -1):
                    emit_p1(st)
                    t_avail.add(st + 1 if st < 7 else 15)
                    pump(6)
                flush(tq)
                t_avail.update(range(NS))
                pump(10 ** 9)
                for st in p3q:
                    emit_p3(st)
            else:
                nc.sync.dma_start(out=wk_t, in_=wk_r)
                nc.scalar.dma_start(out=tri_t, in_=tri_d[:])
                nc.scalar.dma_start(
                    out=wo_t, in_=wo_d[:].rearrange("(t p) o -> p t o", p=128))
                for st in range(NS):
                    emit_p1(st)
                flush(tq)
                for h in range(HL):
                    for hq in range(NQ - 1, -1, -1):
                        emit_pass(h, hq)
                for st in range(NS):
                    emit_p3(st)

    nc.finalize()
    return nc


def rope_tables(S, hd):
    """cos/sin tables matching reference._rope_tables numerics (f32 freqs)."""
    inv = (1.0 / (np.float32(ROPE_BASE) **
                  (np.arange(0, hd, 2, dtype=np.float32) / np.float32(hd))))
    inv = inv.astype(np.float32)
    freqs = (np.arange(S, dtype=np.float32)[:, None] * inv[None, :]
             ).astype(np.float32)
    cos = np.cos(freqs.astype(np.float64)).astype(np.float32)
    sin = np.sin(freqs.astype(np.float64)).astype(np.float32)
    return cos, sin


def make_const_inputs(S):
    """Constant per-core inputs: compact RoPE tables + tri/identity."""
    cos, sin = rope_tables(S, HEAD_DIM)
    bf = ml_dtypes.bfloat16
    return {
        "cs": np.ascontiguousarray(
            np.concatenate([cos, -sin, sin], axis=1).astype(np.float32)),
        "tri": np.triu(np.ones((128, 128), dtype=np.float32)).astype(bf),
        "ident": np.eye(128, dtype=np.float32).astype(bf),
    }


def _is_causal_mask(mask, S):
    m = mask.reshape(S, S)
    rows = np.unique(np.concatenate([np.arange(0, S, S // 64), [S - 1]]))
    for r in rows:
        row = m[r]
        if not np.all(row[:r + 1] == 0.0):
            return False
        if r + 1 < S and not np.all(row[r + 1:] <= -50.0):
            return False
    return True


_NC_CACHE = {}


def kernel(hidden_states, attention_mask, Wqkv, Wo):
    B, S, H = hidden_states.shape
    nh, hd = NUM_HEADS, HEAD_DIM
    HL = nh // (N_CORES // B)       # heads per core
    DL = HL * hd
    G = N_CORES // B                # cores per batch

    if not _is_causal_mask(np.asarray(attention_mask), S):
        return _host_reference(hidden_states, attention_mask, Wqkv, Wo)

    key = (S, H, HL)
    if key not in _NC_CACHE:
        _NC_CACHE[key] = build_nc(S, H, HL)
    nc = _NC_CACHE[key]

    consts = make_const_inputs(S)
    bf = ml_dtypes.bfloat16

    hs = np.asarray(hidden_states, dtype=np.float32)
    Wqkv = np.asarray(Wqkv, dtype=np.float32)
    Wo = np.asarray(Wo, dtype=np.float32)
    hT = [np.ascontiguousarray(hs[b].T) for b in range(B)]

    in_maps = []
    for c in range(N_CORES):
        b, g = divmod(c, G)
        c0 = g * DL
        in_maps.append({
            "hT": hT[b],
            "wq": np.ascontiguousarray(Wqkv[:, c0:c0 + DL]),
            "wk": np.ascontiguousarray(Wqkv[:, H + c0:H + c0 + DL]),
            "wv": np.ascontiguousarray(Wqkv[:, 2 * H + c0:2 * H + c0 + DL]),
            "wo": np.ascontiguousarray(Wo[c0:c0 + DL, :]).astype(bf),
            **consts,
        })

    res = run_bass_kernel_spmd(nc, in_maps, list(range(N_CORES)))
    out = np.empty((B, S, H), dtype=np.float32)
    for b in range(B):
        acc = res.results[b * G]["part"].astype(np.float64)
        for g in range(1, G):
            acc += res.results[b * G + g]["part"].astype(np.float64)
        out[b] = acc.astype(np.float32)
    return out


def _host_reference(hidden_states, attention_mask, Wqkv, Wo):
    """Exact fallback for non-causal masks (numpy, fp32)."""
    B, S, H = hidden_states.shape
    nh, hd = NUM_HEADS, HEAD_DIM
    cos, sin = rope_tables(S, hd)
    qkv = hidden_states.reshape(B * S, H) @ Wqkv
    qkv = qkv.reshape(B, S, 3, nh, hd).transpose(2, 0, 3, 1, 4)
    q, k, v = qkv[0], qkv[1], qkv[2]

    def rope(x):
        x1, x2 = x[..., :hd // 2], x[..., hd // 2:]
        c, s = cos[None, None], sin[None, None]
        return np.concatenate([x1 * c - x2 * s, x2 * c + x1 * s], axis=-1)

    q, k = rope(q), rope(k)
    scores = np.einsum('bhqd,bhkd->bhqk', q, k) * (hd ** -0.5)
    scores = scores + attention_mask.reshape(1, 1, S, S)
    scores -= scores.max(axis=-1, keepdims=True)
    e = np.exp(scores)
    attn = e / e.sum(axis=-1, keepdims=True)
    out = np.einsum('bhqk,bhkd->bhqd', attn, v)
    out = out.transpose(0, 2, 1, 3).reshape(B, S, H)
    return (out @ Wo).astype(np.float32)


# revision 28
# speedup vs baseline: 1.0896x; 1.0615x over previous
"""Causal self-attention (RoPE) Trainium2 Bass kernel, SPMD over 8 NeuronCores.

Sharding: data-parallel over batch (B=2) x tensor-parallel over heads
(16 heads -> 4 heads per core).  core c handles batch c//4, heads
4*(c%4) .. 4*(c%4)+3.  Each core computes its heads' attention output and a
partial out@Wo contribution ([S, H] in f16); the host sums the 4 partials
per batch.

Device pipeline per core (transposed-scores formulation, bf16 attention):
  1. QKV projection (fp32r) from hidden^T; RoPE (f32) on q,k; q,k cast to
     bf16 and PE-transposed (4 tiles into one PSUM bank, single bulk copy);
     v stored natural bf16 with 64 appended ones-columns (V'), so the AV
     matmul replicates the softmax denominator on partitions 64..127 free
     of charge (no partition broadcast needed for normalization).
  2. 512-column q-passes: scores^T[k,q] = K Q^T (bf16) -> exp -> causal tri
     mask on diagonal tiles -> O'^T[0:128, q] = V'^T P~^T accumulated in a
     single PSUM bank per pass; normalization = reciprocal of rows 64:128
     times rows 0:64 (two DVE ops, PSUM-direct).
  3. O^T normalized into on_t (bf16) -> partial = O_norm @ Wo (bf16) in
     512-wide chunks -> f16 partial out, DMA'd on the DVE queue.
"""

import sys
import numpy as np

for _p in ("/opt/trn_rl_repo", "/root/.axon_site/_ro/trn_rl_repo"):
    if _p not in sys.path:
        sys.path.append(_p)

import ml_dtypes
import concourse.bacc as bacc
from concourse import mybir
from concourse.tile import TileContext
from concourse.bass_utils import run_bass_kernel_spmd

F32 = mybir.dt.float32
F32R = mybir.dt.float32r
BF16 = mybir.dt.bfloat16
F16 = mybir.dt.float16
EXP = mybir.ActivationFunctionType.Exp

NUM_HEADS = 16
HEAD_DIM = 64
ROPE_BASE = 160000.0
N_CORES = 8


def build_nc(S, H, HL, debug=False):
    """Build the SPMD Bass program.

    S: sequence length; H: hidden size; HL: heads per core (local).
    """
    DL = HL * HEAD_DIM          # local channels (256)
    NI = H // 128               # contraction tiles for projections (8)
    NS = S // 128               # sequence tiles (16)
    CT = max(DL // 128, 1)      # channel tiles (2)
    NQ = S // 512               # 512-wide q passes per head (4)
    scale = HEAD_DIM ** -0.5
    assert DL % 128 == 0 and H % 128 == 0 and S % 1024 == 0

    nc = bacc.Bacc("TRN2", target_bir_lowering=False, debug=False,
                   num_devices=N_CORES)
    hT_d = nc.declare_dram_parameter("hT", [H, S], BF16, isOutput=False)
    wq_d = nc.declare_dram_parameter("wq", [H, DL], BF16, isOutput=False)
    wk_d = nc.declare_dram_parameter("wk", [H, DL], BF16, isOutput=False)
    wv_d = nc.declare_dram_parameter("wv", [H, DL], BF16, isOutput=False)
    wo_d = nc.declare_dram_parameter("wo", [DL, H], BF16, isOutput=False)
    cs_d = nc.declare_dram_parameter("cs", [128, S // 128, 96], F32,
                                     isOutput=False)
    tri_d = nc.declare_dram_parameter("tri", [128, 128], BF16, isOutput=False)
    id_d = nc.declare_dram_parameter("ident", [128, 128], BF16, isOutput=False)
    out_d = nc.declare_dram_parameter("part", [S, H], F16, isOutput=True)

    with TileContext(nc) as tc:
        with (
            tc.tile_pool(name="w", bufs=1) as w_pool,
            tc.tile_pool(name="persist", bufs=1) as pers,
            tc.tile_pool(name="hstream", bufs=4) as hs_pool,
            tc.tile_pool(name="xall", bufs=3) as xa_pool,
            tc.tile_pool(name="rope", bufs=3) as rope_pool,
            tc.tile_pool(name="qksb", bufs=3) as qk_pool,
            tc.tile_pool(name="psb", bufs=6) as p_pool,
            tc.tile_pool(name="norm", bufs=3) as n_pool,
            tc.tile_pool(name="osb", bufs=3) as o_pool,
            tc.tile_pool(name="ps_sc", bufs=3, space="PSUM") as ps_sc,
            tc.tile_pool(name="ps_o", bufs=1, space="PSUM") as ps_o,
            tc.tile_pool(name="ps_x", bufs=3, space="PSUM") as ps_x,
        ):
            # --- weights / constants (resident) ---
            wq_t = w_pool.tile([128, NI, DL], BF16)
            wk_t = w_pool.tile([128, NI, DL], BF16)
            wv_t = w_pool.tile([128, NI, DL], BF16)
            wo_t = w_pool.tile([128, CT, H], BF16)
            cs_t = w_pool.tile([128, NS, 96], F32)
            tri_t = w_pool.tile([128, 128], BF16)
            id_t = w_pool.tile([128, 128], BF16)

            NIH = NI // 2
            # weight streams split across both HWDGE queues so the first
            # projection groups aren't gated on one serialized queue:
            # ACT carries wq/wv (q and v groups run first), SP carries wk
            # (behind the first h tile).  tri/wo are deferred below.
            wq_r = wq_d[:].rearrange("(t p) d -> p t d", p=128)
            wv_r = wv_d[:].rearrange("(t p) d -> p t d", p=128)
            wk_r = wk_d[:].rearrange("(t p) d -> p t d", p=128)
            nc.scalar.dma_start(out=wq_t[:, 0:NIH, :], in_=wq_r[:, 0:NIH, :])
            nc.scalar.dma_start(out=wq_t[:, NIH:, :], in_=wq_r[:, NIH:, :])
            nc.scalar.dma_start(out=wv_t[:, 0:NIH, :], in_=wv_r[:, 0:NIH, :])
            nc.scalar.dma_start(out=wv_t[:, NIH:, :], in_=wv_r[:, NIH:, :])
            nc.scalar.dma_start(out=cs_t, in_=cs_d[:])
            nc.scalar.dma_start(out=id_t, in_=id_d[:])

            # persistent activations
            qkT = pers.tile([128, 2, CT, S], BF16)   # [d, q/k, ct, s]
            vv = pers.tile([128, NS, HL, 128], BF16)  # v cols 0:64, ones 64:128
            on_t = pers.tile([128, CT, S], BF16)
            nc.gpsimd.memset(vv[:, :, :, 64:128], 1.0)

            hT_r = hT_d[:].rearrange("(t p) s -> p t s", p=128)

            # ---------------- deferred-emission queues ----------------
            tq = []   # pending transpose closures (phase 1)
            avq = []  # pending AV closures (phase 2)

            def flush(qu, keep=0):
                while len(qu) > keep:
                    qu.pop(0)()

            # ---------------- phase 2 ----------------
            # two PSUM o-accumulator slots, handed out as free tags so a
            # pass always reuses the slot of the LAST pass to finish (a
            # fixed ring would reallocate a still-accumulating slot)
            o_tiles = {}
            free_otags = ["oA", "oB"]

            def seg_for(h, t, hq, first=False, last=False):
                """scores+exp+mask for k-tile t against q in
                [max(128t, 512*hq), 512*(hq+1)); AV deferred."""
                sg = hq * 512
                k0 = t * 128
                o0 = max(k0 - sg, 0)
                if first:
                    assert o0 == 0, "first seg of a pass must cover the q-range"
                base = (h % 2) * 64
                ct = h // 2
                kT = qkT[base:base + 64, 1, ct, k0:k0 + 128]
                qT = qkT[base:base + 64, 0, ct, sg + o0:sg + 512]
                if (h, hq) not in o_tiles:
                    otag = free_otags.pop(0)
                    o_tiles[(h, hq)] = (
                        ps_o.tile([128, 512], F32, tag=otag,
                                  name=f"o_{h}_{hq}"), otag)
                o_ps = o_tiles[(h, hq)][0]
                sc = ps_sc.tile([128, 512], F32, tag="sc")
                nc.tensor.matmul(sc[:, o0:512], kT, qT, start=True, stop=True)
                flush(avq, keep=3)
                p = p_pool.tile([128, 512], BF16, tag="p")
                nc.scalar.activation(p[:, o0:512], sc[:, o0:512], EXP,
                                     scale=scale)
                if k0 >= sg:  # diagonal tile: causal mask
                    nc.vector.tensor_mul(p[:, o0:o0 + 128], p[:, o0:o0 + 128],
                                         tri_t)

                def av():
                    nc.tensor.matmul(o_ps[:, o0:512], vv[:, t, h, :],
                                     p[:, o0:512], start=first, stop=last,
                                     skip_group_check=True)
                avq.append(av)

            def norm_for(h, hq):
                flush(avq)
                sg = hq * 512
                base = (h % 2) * 64
                ct = h // 2
                o_ps, otag = o_tiles.pop((h, hq))
                free_otags.append(otag)
                # rows 64:128 of O' hold the softmax denominator replicated
                # (ones-columns of V'); normalize PSUM-direct: two DVE ops
                r_sb = n_pool.tile([64, 512], F32, tag="r")
                nc.vector.reciprocal(r_sb, o_ps[64:128, :])
                nc.vector.tensor_mul(on_t[base:base + 64, ct, sg:sg + 512],
                                     o_ps[0:64, :], r_sb)

            # ---------------- phase 1 ----------------
            # h streams in double-s-tiles ([128, NI, 256] bf16) so each DMA
            # descriptor stays >= 512B (dodges the small-element 2x DMA
            # latency penalty); each load feeds two emit_p1 calls
            h_cache = {}

            def load_h(pair, split_h=False, after_h=None):
                s0 = pair * 256
                h_t = hs_pool.tile([128, NI, 256], BF16, tag="h")
                if split_h:
                    nc.sync.dma_start(out=h_t[:, 0:NIH, :],
                                      in_=hT_r[:, 0:NIH, s0:s0 + 256])
                    if after_h is not None:
                        after_h()
                    nc.sync.dma_start(out=h_t[:, NIH:, :],
                                      in_=hT_r[:, NIH:, s0:s0 + 256])
                else:
                    nc.sync.dma_start(out=h_t, in_=hT_r[:, :, s0:s0 + 256])
                    if after_h is not None:
                        after_h()
                h_cache[pair] = h_t

            def emit_p1(st):
                s0 = st * 128
                pair, off = st // 2, (st % 2) * 128
                h_t = h_cache[pair]
                qk_ps = ps_x.tile([128, 512], F32, tag="px")
                v_ps = ps_x.tile([128, 512], F32, tag="px")
                for w_t, ps, col in ((wq_t, qk_ps, 0), (wv_t, v_ps, 0),
                                     (wk_t, qk_ps, 256)):
                    for i in range(NI):
                        nc.tensor.matmul(ps[:, col:col + DL],
                                         h_t[:, i, off:off + 128],
                                         w_t[:, i, :],
                                         start=(i == 0), stop=(i == NI - 1))
                flush(tq)

                # drain on DVE+Pool only: the ACT queue stays exp-only, so
                # phase-1 never stalls behind weight-DMA issues or exps
                x_all = xa_pool.tile([128, 512], F32, tag="xa")
                nc.vector.tensor_copy(x_all, qk_ps)
                nc.gpsimd.tensor_copy(
                    vv[:, st, :, 0:64],
                    v_ps[:, 0:DL].rearrange("p (h d) -> p h d", d=64))

                cosb = cs_t[:, st, 0:32].unsqueeze(1).broadcast_to(
                    [128, 2 * HL, 32])
                sinmb = cs_t[:, st, 32:64].unsqueeze(1).broadcast_to(
                    [128, HL, 32])
                sinpb = cs_t[:, st, 64:96].unsqueeze(1).broadcast_to(
                    [128, HL, 32])
                x_sb = {}
                for qk in (0, 1):
                    xsrc = x_all[:, qk * DL:(qk + 1) * DL]
                    x4 = xsrc.rearrange("p (h two d) -> p h two d", two=2,
                                        d=32)
                    a_t = rope_pool.tile([128, DL], F32, tag="ra")
                    nc.vector.tensor_mul(
                        a_t.rearrange("p (r d) -> p r d", d=32),
                        xsrc.rearrange("p (r d) -> p r d", d=32), cosb)
                    b_t = rope_pool.tile([128, DL], F32, tag="rb")
                    b4 = b_t.rearrange("p (h two d) -> p h two d", two=2,
                                       d=32)
                    nc.vector.tensor_mul(b4[:, :, 0, :], x4[:, :, 1, :],
                                         sinmb)
                    nc.vector.tensor_mul(b4[:, :, 1, :], x4[:, :, 0, :],
                                         sinpb)
                    xs = qk_pool.tile([128, DL], BF16, tag=f"x{qk}")
                    nc.gpsimd.tensor_add(xs, a_t, b_t)
                    x_sb[qk] = xs

                def transposes():
                    t_ps = ps_x.tile([128, 1024], BF16, tag="px")
                    for qk in (0, 1):
                        for ctp in range(CT):
                            idx = qk * CT + ctp
                            nc.tensor.transpose(
                                t_ps[:, idx * 128:(idx + 1) * 128],
                                x_sb[qk][:, ctp * 128:(ctp + 1) * 128],
                                id_t)
                    nc.vector.tensor_copy(
                        qkT[:, :, :, s0:s0 + 128],
                        t_ps[:, 0:512].rearrange("p (a b s) -> p a b s",
                                                 a=2, b=CT))
                tq.append(transposes)

            # ---------------- phase 3 ----------------
            def emit_p3(st):
                s0 = st * 128
                out_sb = o_pool.tile([128, H], F16, tag="out")
                for oc, ceng in ((0, nc.gpsimd), (512, nc.vector)):
                    ps = ps_x.tile([128, 512], F32, tag="px")
                    for ctp in range(CT):
                        nc.tensor.matmul(ps, on_t[:, ctp, s0:s0 + 128],
                                         wo_t[:, ctp, oc:oc + 512],
                                         start=(ctp == 0),
                                         stop=(ctp == CT - 1))
                    ceng.tensor_copy(out_sb[:, oc:oc + 512], ps)
                nc.sync.dma_start(out=out_d[s0:s0 + 128, :], in_=out_sb)

            # ---------------- orchestration ----------------
            def pass_k_order(hq):
                first = 4 * hq
                return ([first] + list(range(first + 1, 4 * (hq + 1))) +
                        list(range(first - 1, -1, -1)))

            def emit_pass(h, hq):
                ts = pass_k_order(hq)
                for j, t in enumerate(ts):
                    seg_for(h, t, hq, first=(j == 0), last=(j == len(ts) - 1))
                norm_for(h, hq)

            if NS == 16:
                # global pass scheduler: 16 passes processed in order, segs
                # emitted as their k-tiles' transposes land; a pass may open
                # on ANY k-tile with t <= 4*hq (full q coverage).  P3 tiles
                # unlock when their q-tier's norms are all emitted and get
                # wedged one per norm to fill pass-transition bubbles.
                t_avail = set()
                passes = [{"h": h, "hq": hq, "rem": pass_k_order(hq),
                           "started": False}
                          for hq in (3, 2, 1, 0) for h in range(HL)]
                tier_left = {hq: HL for hq in range(NQ)}
                p3q = []

                def pump_one(pst, budget):
                    """Emit up to budget ready segs from one pass; returns
                    (emitted, finished)."""
                    hq = pst["hq"]
                    if pst["started"]:
                        ready = [t for t in pst["rem"] if t in t_avail]
                    elif not free_otags:
                        return 0, False   # no accumulator slot free yet
                    else:
                        ready = [t for t in pst["rem"]
                                 if t in t_avail and t <= 4 * hq]
                    n = min(budget, len(ready))
                    fin = False
                    for t in ready[:n]:
                        is_first = not pst["started"]
                        pst["started"] = True
                        pst["rem"].remove(t)
                        is_last = not pst["rem"]
                        seg_for(pst["h"], t, hq, first=is_first, last=is_last)
                        if is_last:
                            norm_for(pst["h"], hq)
                            tier_left[hq] -= 1
                            if tier_left[hq] == 0:
                                p3q.extend(range(4 * hq, 4 * hq + 4))
                            if p3q:
                                emit_p3(p3q.pop(0))
                            fin = True
                    return n, fin

                def pump(budget):
                    # two o-accumulator slots -> at most two passes in
                    # flight; drain the front pass first
                    while budget > 0 and passes:
                        n0, fin = pump_one(passes[0], budget)
                        if fin:
                            passes.pop(0)
                            budget -= n0
                            continue
                        budget -= n0
                        if budget <= 0 or len(passes) < 2:
                            return budget
                        n1, fin1 = pump_one(passes[1], budget)
                        if fin1:
                            passes.pop(1)
                        budget -= n1
                        if n0 == 0 and n1 == 0:
                            return budget
                    return budget

                # upper P1 (s-tiles 8..15), h in double-tiles, wk on the SP
                # queue wedged into the first h load; tri/wo deferred
                def load_wk():
                    nc.sync.dma_start(out=wk_t[:, 0:NIH, :],
                                      in_=wk_r[:, 0:NIH, :])
                    nc.sync.dma_start(out=wk_t[:, NIH:, :],
                                      in_=wk_r[:, NIH:, :])
                load_h(4, split_h=True, after_h=load_wk)
                load_h(5)
                nc.scalar.dma_start(out=tri_t, in_=tri_d[:])
                upper = [8, 9, 10, 11, 12, 13, 14, 15]
                prefetch_u = {10: 6, 12: 7, 14: 3}
                for st in upper:
                    if st in prefetch_u:
                        load_h(prefetch_u[st])
                    emit_p1(st)   # flushes T(st-1)
                    if st > 8:
                        t_avail.add(st - 1)
                    pump(3)
                nc.scalar.dma_start(
                    out=wo_t, in_=wo_d[:].rearrange("(t p) o -> p t o", p=128))
                prefetch_l = {7: 2, 5: 1, 3: 0}
                for st in range(7, -1, -1):
                    if st in prefetch_l:
                        load_h(prefetch_l[st])
                    emit_p1(st)
                    t_avail.add(st + 1 if st < 7 else 15)
                    pump(6)
                flush(tq)
                t_avail.update(range(NS))
                pump(10 ** 9)
                for st in p3q:
                    emit_p3(st)
            else:
                nc.sync.dma_start(out=wk_t, in_=wk_r)
                nc.scalar.dma_start(out=tri_t, in_=tri_d[:])
                nc.scalar.dma_start(
                    out=wo_t, in_=wo_d[:].rearrange("(t p) o -> p t o", p=128))
                for st in range(NS):
                    if st % 2 == 0:
                        load_h(st // 2)
                    emit_p1(st)
                flush(tq)
                for h in range(HL):
                    for hq in range(NQ - 1, -1, -1):
                        emit_pass(h, hq)
                for st in range(NS):
                    emit_p3(st)

    nc.finalize()
    return nc


def rope_tables(S, hd):
    """cos/sin tables matching reference._rope_tables numerics (f32 freqs)."""
    inv = (1.0 / (np.float32(ROPE_BASE) **
                  (np.arange(0, hd, 2, dtype=np.float32) / np.float32(hd))))
    inv = inv.astype(np.float32)
    freqs = (np.arange(S, dtype=np.float32)[:, None] * inv[None, :]
             ).astype(np.float32)
    cos = np.cos(freqs.astype(np.float64)).astype(np.float32)
    sin = np.sin(freqs.astype(np.float64)).astype(np.float32)
    return cos, sin


def make_const_inputs(S):
    """Constant per-core inputs: compact RoPE tables + tri/identity."""
    cos, sin = rope_tables(S, HEAD_DIM)
    bf = ml_dtypes.bfloat16
    cs = np.concatenate([cos, -sin, sin], axis=1).astype(np.float32)  # [S,96]
    cs = cs.reshape(S // 128, 128, 96).transpose(1, 0, 2)  # [128, NS, 96]
    return {
        "cs": np.ascontiguousarray(cs),
        "tri": np.triu(np.ones((128, 128), dtype=np.float32)).astype(bf),
        "ident": np.eye(128, dtype=np.float32).astype(bf),
    }


def _is_causal_mask(mask, S):
    m = mask.reshape(S, S)
    rows = np.unique(np.concatenate([np.arange(0, S, S // 64), [S - 1]]))
    for r in rows:
        row = m[r]
        if not np.all(row[:r + 1] == 0.0):
            return False
        if r + 1 < S and not np.all(row[r + 1:] <= -50.0):
            return False
    return True


_NC_CACHE = {}


def kernel(hidden_states, attention_mask, Wqkv, Wo):
    B, S, H = hidden_states.shape
    nh, hd = NUM_HEADS, HEAD_DIM
    HL = nh // (N_CORES // B)       # heads per core
    DL = HL * hd
    G = N_CORES // B                # cores per batch

    if not _is_causal_mask(np.asarray(attention_mask), S):
        return _host_reference(hidden_states, attention_mask, Wqkv, Wo)

    key = (S, H, HL)
    if key not in _NC_CACHE:
        _NC_CACHE[key] = build_nc(S, H, HL)
    nc = _NC_CACHE[key]

    consts = make_const_inputs(S)
    bf = ml_dtypes.bfloat16

    hs = np.asarray(hidden_states, dtype=np.float32)
    Wqkv = np.asarray(Wqkv, dtype=np.float32)
    Wo = np.asarray(Wo, dtype=np.float32)
    hT = [np.ascontiguousarray(hs[b].T).astype(bf) for b in range(B)]

    in_maps = []
    for c in range(N_CORES):
        b, g = divmod(c, G)
        c0 = g * DL
        in_maps.append({
            "hT": hT[b],
            "wq": np.ascontiguousarray(Wqkv[:, c0:c0 + DL]).astype(bf),
            "wk": np.ascontiguousarray(Wqkv[:, H + c0:H + c0 + DL]).astype(bf),
            "wv": np.ascontiguousarray(
                Wqkv[:, 2 * H + c0:2 * H + c0 + DL]).astype(bf),
            "wo": np.ascontiguousarray(Wo[c0:c0 + DL, :]).astype(bf),
            **consts,
        })

    res = run_bass_kernel_spmd(nc, in_maps, list(range(N_CORES)))
    out = np.empty((B, S, H), dtype=np.float32)
    for b in range(B):
        acc = res.results[b * G]["part"].astype(np.float64)
        for g in range(1, G):
            acc += res.results[b * G + g]["part"].astype(np.float64)
        out[b] = acc.astype(np.float32)
    return out


def _host_reference(hidden_states, attention_mask, Wqkv, Wo):
    """Exact fallback for non-causal masks (numpy, fp32)."""
    B, S, H = hidden_states.shape
    nh, hd = NUM_HEADS, HEAD_DIM
    cos, sin = rope_tables(S, hd)
    qkv = hidden_states.reshape(B * S, H) @ Wqkv
    qkv = qkv.reshape(B, S, 3, nh, hd).transpose(2, 0, 3, 1, 4)
    q, k, v = qkv[0], qkv[1], qkv[2]

    def rope(x):
        x1, x2 = x[..., :hd // 2], x[..., hd // 2:]
        c, s = cos[None, None], sin[None, None]
        return np.concatenate([x1 * c - x2 * s, x2 * c + x1 * s], axis=-1)

    q, k = rope(q), rope(k)
    scores = np.einsum('bhqd,bhkd->bhqk', q, k) * (hd ** -0.5)
    scores = scores + attention_mask.reshape(1, 1, S, S)
    scores -= scores.max(axis=-1, keepdims=True)
    e = np.exp(scores)
    attn = e / e.sum(axis=-1, keepdims=True)
    out = np.einsum('bhqk,bhkd->bhqd', attn, v)
    out = out.transpose(0, 2, 1, 3).reshape(B, S, H)
    return (out @ Wo).astype(np.float32)
